# revision 1
# baseline (speedup 1.0000x reference)
"""ConvGRU Trainium2 kernel.

Full inputs -> 8-core SPMD Bass kernel -> full output.

Sharding: 8 cores = 4 batch elements x 2 H-halves. Each core owns 32 rows
of H and redundantly computes a "growing halo" (rows 32..32+e, e = T-t)
so the T=16 recurrence needs no cross-core communication. Bottom halves
are vertically flipped on the host (x rows and W ky taps) so every core
runs the identical program: owned rows 0..31, halo extending downward,
zero-pad above row 0.

Per step t the fused conv gi+gh is computed as 9 (ky,kx) taps of
K=96-stacked matmuls: rhs partitions 0..31 = x_t, 32..95 = h_{t-1},
both bf16 in one padded buffer [96, 49, 66]. Two M=128 weight matrices
per tap:
  w1 cols 0..63  = i_n  (x rows only, h rows zero)
  w1 cols 64..127 = r-gate (full 96-K)
  w2 cols 0..63  = h_n  (h rows only, x rows zero)
  w2 cols 64..127 = z-gate (full 96-K)
PSUM p1 = [i_n | r_pre], p2 = [h_n | z_pre]. Elementwise GRU math runs
on partitions 64..127 (PSUM operands may be read at a different base
partition than SBUF operands; SBUF-SBUF pairs stay aligned).
"""

import os
import sys

sys.path.insert(0, "/opt/trn_rl_repo")

import numpy as np

T, CIN, HID, H, W = 16, 32, 64, 64, 64
B = 4
NCORES = 8
OWN = 32           # owned H rows per core
XR = 48            # x slice rows fed to each core (owned + max halo + 1)
PR = 49            # padded rows: index p = unpadded row + 1, rows -1..47
PC = 66            # padded cols: index c = unpadded col + 1
CHUNK = 8          # output rows per chunk (8*64 = 512 = one PSUM bank)

_CACHE = {}
KERNEL_STATS = {}


def _n_rows(t):
    # valid h_t rows needed: owned + halo that future steps consume
    return OWN + (T - t)


def _build():
    import concourse.bacc as bacc
    import concourse.mybir as mybir
    from concourse import tile

    dt = mybir.dt
    AF = mybir.ActivationFunctionType

    nc = bacc.Bacc("TRN2", target_bir_lowering=False, debug=False,
                   num_devices=NCORES)
    xin = nc.dram_tensor("xin", [T, CIN, XR, W], dt.bfloat16,
                         kind="ExternalInput")
    w1 = nc.dram_tensor("w1", [96, 9 * 128], dt.bfloat16,
                        kind="ExternalInput")
    w2 = nc.dram_tensor("w2", [96, 9 * 128], dt.bfloat16,
                        kind="ExternalInput")
    out = nc.dram_tensor("out", [T, HID, OWN, W], dt.float32,
                         kind="ExternalOutput")

    with tile.TileContext(nc) as tc:
        with tc.tile_pool(name="const", bufs=1) as const, \
             tc.tile_pool(name="state", bufs=1) as state, \
             tc.tile_pool(name="work", bufs=3) as work, \
             tc.tile_pool(name="psum", bufs=4, space="PSUM") as psum:

            w1s = const.tile([96, 9 * 128], dt.bfloat16, tag="w1s")
            w2s = const.tile([96, 9 * 128], dt.bfloat16, tag="w2s")
            nc.sync.dma_start(w1s[:], w1[:])
            nc.sync.dma_start(w2s[:], w2[:])

            # stacked conv-input buffers, rotated mod 3 so x_{t+1} loads
            # never wait on step-t reads. x on partitions 0..31, h on 32..95.
            sb = [state.tile([96, PR * PC], dt.bfloat16, tag=f"sb{i}",
                             name=f"sb{i}")
                  for i in range(3)]
            for i in range(3):
                nc.gpsimd.memset(sb[i][:], 0.0)
            sb3 = [s.rearrange("p (r c) -> p r c", c=PC) for s in sb]

            # f32 hidden state (upper 64 partitions), ping-pong
            NH = _n_rows(1)  # 47
            hf = [state.tile([128, NH * W], dt.float32, tag=f"hf{i}",
                             name=f"hf{i}")
                  for i in range(2)]
            nc.vector.memset(hf[0][64:128, :], 0.0)

            for t in range(1, T + 1):
                nt = _n_rows(t)
                cur = sb3[(t - 1) % 3]
                nxt = sb3[t % 3]
                hprev = hf[(t - 1) % 2]
                hcur = hf[t % 2]

                if t == 1:
                    # x_1 + x_2: later x_{t+1} loads are issued during step t
                    nc.sync.dma_start(cur[0:32, 1:1 + XR, 1:1 + W], xin[0])
                if t < T:
                    nc.sync.dma_start(nxt[0:32, 1:1 + XR, 1:1 + W], xin[t])

                nchunks = (nt + CHUNK - 1) // CHUNK
                for ci in range(nchunks):
                    r0 = ci * CHUNK
                    nr = min(CHUNK, nt - r0)
                    N = nr * W
                    p1 = psum.tile([128, CHUNK * W], dt.float32, tag="p1")
                    p2 = psum.tile([128, CHUNK * W], dt.float32, tag="p2")
                    for ws, pp in ((w1s, p1), (w2s, p2)):
                        for tap in range(9):
                            ky, kx = divmod(tap, 3)
                            rhs = cur[0:96, r0 + ky:r0 + ky + nr, kx:kx + W]
                            nc.tensor.matmul(
                                pp[:, :N],
                                ws[:, tap * 128:(tap + 1) * 128],
                                rhs, start=(tap == 0), stop=(tap == 8))

                    r_s = work.tile([128, CHUNK * W], dt.float32, tag="r_s")
                    z_s = work.tile([128, CHUNK * W], dt.float32, tag="z_s")
                    nc.scalar.activation(r_s[64:128, :N], p1[64:128, :N],
                                         AF.Sigmoid)
                    nc.scalar.activation(z_s[64:128, :N], p2[64:128, :N],
                                         AF.Sigmoid)
                    t_rn = work.tile([128, CHUNK * W], dt.float32, tag="t_rn")
                    # r * h_n  (SBUF upper x PSUM lower)
                    nc.vector.tensor_mul(t_rn[64:128, :N], r_s[64:128, :N],
                                         p2[0:64, :N])
                    # + i_n
                    nc.vector.tensor_add(t_rn[64:128, :N], t_rn[64:128, :N],
                                         p1[0:64, :N])
                    n_t = work.tile([128, CHUNK * W], dt.float32, tag="n_t")
                    nc.scalar.activation(n_t[64:128, :N], t_rn[64:128, :N],
                                         AF.Tanh)
                    d_t = work.tile([128, CHUNK * W], dt.float32, tag="d_t")
                    nc.vector.tensor_sub(d_t[64:128, :N],
                                         hprev[64:128, r0 * W:r0 * W + N],
                                         n_t[64:128, :N])
                    nc.vector.tensor_mul(d_t[64:128, :N], z_s[64:128, :N],
                                         d_t[64:128, :N])
                    # h_new = n + z*(h - n) -> persistent f32 state
                    nc.vector.tensor_add(hcur[64:128, r0 * W:r0 * W + N],
                                         n_t[64:128, :N], d_t[64:128, :N])
                    # bf16 convert for the next conv input
                    hb = work.tile([128, CHUNK * W], dt.bfloat16, tag="hb")
                    nc.vector.tensor_copy(hb[64:128, :N],
                                          hcur[64:128, r0 * W:r0 * W + N])
                    src_b = hb[64:128, :N].rearrange("p (r c) -> p r c", c=W)
                    nc.sync.dma_start(nxt[32:96, 1 + r0:1 + r0 + nr, 1:1 + W],
                                      src_b)
                    if r0 < OWN:
                        src_f = hcur[64:128, r0 * W:r0 * W + N].rearrange(
                            "p (r c) -> p r c", c=W)
                        nc.sync.dma_start(out[t - 1, :, r0:r0 + nr, :], src_f)

    nc.compile()
    return nc


def _prep_inputs(x, W_i, W_h):
    import ml_dtypes

    bf16 = ml_dtypes.bfloat16
    in_maps = []
    for c in range(NCORES):
        b, half = divmod(c, 2)
        xs = x[b]                      # [T, CIN, H, W]
        Wi, Wh = W_i, W_h
        if half == 1:
            xs = xs[:, :, ::-1, :]
            Wi = W_i[:, :, ::-1, :]
            Wh = W_h[:, :, ::-1, :]
        xs = np.ascontiguousarray(xs[:, :, :XR, :]).astype(bf16)

        w1 = np.zeros((9, 96, 128), np.float32)
        w2 = np.zeros((9, 96, 128), np.float32)
        for tap in range(9):
            ky, kx = divmod(tap, 3)
            # w1: cols 0..63 = i_n (x rows), cols 64..127 = r gate (full)
            w1[tap, 0:32, 0:64] = Wi[128:192, :, ky, kx].T
            w1[tap, 0:32, 64:128] = Wi[0:64, :, ky, kx].T
            w1[tap, 32:96, 64:128] = Wh[0:64, :, ky, kx].T
            # w2: cols 0..63 = h_n (h rows), cols 64..127 = z gate (full)
            w2[tap, 32:96, 0:64] = Wh[128:192, :, ky, kx].T
            w2[tap, 0:32, 64:128] = Wi[64:128, :, ky, kx].T
            w2[tap, 32:96, 64:128] = Wh[64:128, :, ky, kx].T
        # DRAM layout [96, 9*128]
        w1 = np.ascontiguousarray(w1.transpose(1, 0, 2).reshape(96, 9 * 128))
        w2 = np.ascontiguousarray(w2.transpose(1, 0, 2).reshape(96, 9 * 128))
        in_maps.append({"xin": xs, "w1": w1.astype(bf16),
                        "w2": w2.astype(bf16)})
    return in_maps


def kernel(x, W_i, W_h):
    from concourse.bass_utils import run_bass_kernel_spmd

    x = np.asarray(x, dtype=np.float32)
    W_i = np.asarray(W_i, dtype=np.float32)
    W_h = np.asarray(W_h, dtype=np.float32)

    if "nc" not in _CACHE:
        _CACHE["nc"] = _build()
    nc = _CACHE["nc"]

    in_maps = _prep_inputs(x, W_i, W_h)
    trace = bool(os.environ.get("BASS_TRACE"))
    res = run_bass_kernel_spmd(nc, in_maps, list(range(NCORES)), trace=trace)
    KERNEL_STATS["exec_time_ns"] = res.exec_time_ns
    KERNEL_STATS["trace"] = res.instructions_and_trace

    y = np.empty((B, T, HID, H, W), np.float32)
    for c in range(NCORES):
        b, half = divmod(c, 2)
        oc = res.results[c]["out"]     # [T, HID, OWN, W]
        if half == 0:
            y[b, :, :, 0:OWN, :] = oc
        else:
            y[b, :, :, OWN:H, :] = oc[:, :, ::-1, :]
    return y

